# revision 24
# baseline (speedup 1.0000x reference)
"""TRN2 Bass kernel for nn_ComboFwdVecComp (B=4, S=512, C=V=128).

out[b,i,j,v] = tanh( sum_c ctx[b,i,c]*ctx[b,j,c]*Wm[v,c]        (M term)
                     + ctx[b,i,:] @ (W2-Wd).T                    (i-dep, folded in rhs)
                     + ctx[b,j,:] @ (W1+Wd).T + (b1+b2+bm+bd)    (arow, j-dep K=1 mms) )

Output (4,512,512,128) f32 = 512 MiB -> memory-bound (HBM write dominated).

Sharding: 8 cores, core k handles b = k//2, j in [ (k%2)*256, +256 ), ALL i.
Each core emits out_shard (512, 256, 128) = 64 MiB; host concatenates on j.

Layout: psum/out partitions = i, free dims = (j, v) which are CONTIGUOUS in
HBM. Each store DMA is [128 i, 16 j x 128 v] = 1 MiB with 8 KiB contiguous
per partition (128 descriptors of 8 KiB). A partitions=j layout stores 512 B
granules and caps at ~233 GB/s (descriptor-rate bound); this one runs at the
SDMA line rate (~420 GB/s when fed).

Why shard j (not i) across the core pair: the DVE rhs-prep
rhs'[c,(j,v)] = WmT[c,v]*ctxj[c] + W2dT[c,v] depends only on j, and is
reused by every i-block. With 256 j's and 4 i-blocks of 128 per core, each
prep is consumed 4x, so DVE does ~91 us of prep per core -- safely under
the ~165 us DMA floor. (An i-sharded core pair preps all 512 j's for only
2 i-blocks = 2x the DVE work, and DVE was becoming the bottleneck.)
Prep must stay on ONE elementwise engine: DVE 2-source ops and ANY GpSimd
op arbitrate an exclusive SBUF shared-port lock and fully serialize.

Per-core structure: j is processed in 8 jblocks of 32 j's (8 quads of 4).
PSUM is one [128, 4096] megatile (8 banks); bank = (half, s) = one j-quad.
Per jblock: DVE preps rhs' once (two [C,2048] mult+add pairs, f32r out),
then ALL FOUR i-blocks consume it:
  8 bias mms (K=1, N=512) first: ones^T @ arowp -> bank, strip-tiled on PE
    row-strips 0/32/64/96 (4 run concurrently), two groups;
  8 main mms (K=128, N=512) after, ONE ctxT LDW for all 8: ctxT_chunk_ib^T
    @ rhs'_quad accumulates on the bias. Long PE bursts + few LDWs keep the
    PE HAM clock warm.
  ACT tanh drains each half [128,2048] -> SBUF as soon as its 4 mains are
  done; ONE 1 MiB DMA per half stores it, alternating SP/ACT HWDGE queues
  (gpsimd SWDGE would get lock-blocked by DVE preps).

All matmuls run in float32r (TF32-like, ~1.5e-4 rel err, ~1 cyc/row at
N=512; plain fp32 is 4 cyc/row). fp32r operands must come from a rounding
compute op, so ctxT/ones/arowp are rounded by DVE copies and rhs' by its
producing DVE add.
arowp rows live on partitions {0,32,64,96}: j-quad q -> partition (q%4)*32,
column block q//4 (K=1 matmul base rules + strip tiling). arowp rows are
issued FIRST on both HWDGE queues (the Tile scheduler bakes its modeled DMA
completion order into semaphore waits).
"""

import sys
import types
from contextlib import ExitStack

import numpy as np

import concourse.bass as bass
import concourse.mybir as mybir
import concourse.tile as tile
from concourse import bacc
from concourse.bass_utils import run_bass_kernel_spmd

B, S, C, V = 4, 512, 128, 128
NCORES = 8
NJ = 256          # j's per core
JQ = NJ // 4      # j-quads per core (64)
NJB = 8           # jblocks (8 j-quads = 32 j's each)
NIB = 4           # i-blocks of 128 partitions (all of S)

_F32 = mybir.dt.float32
_F32R = mybir.dt.float32r
_BF16 = mybir.dt.bfloat16
_FP16 = mybir.dt.float16


def install_ntff_shim():
    """antenv.axon_hooks is absent on some images; shim it so trace=True works."""
    if "antenv.axon_hooks" in sys.modules:
        return
    try:
        from trn_agent_boot.trn_boot import _ntff_profile_via_ctypes
        hook = _ntff_profile_via_ctypes("/opt/axon/libaxon_pjrt.so")
    except Exception:
        hook = None
    mod = types.ModuleType("antenv.axon_hooks")
    mod.get_axon_ntff_profile_hook = lambda: hook
    mod.set_axon_ntff_profile_hook = lambda h: None
    sys.modules["antenv.axon_hooks"] = mod


def build_nc():
    nc = bacc.Bacc("TRN2", target_bir_lowering=False, debug=False)

    ctxT_d = nc.dram_tensor("ctxT", [C, S], _F32, kind="ExternalInput").ap()
    # ctxjT | wmT | w2dT packed in one tensor -> ONE load DMA (each separate
    # HWDGE load pays ~2 us completion latency serially on its ring, and the
    # first DVE prep was stuck ~6 us behind three of them)
    consts_d = nc.dram_tensor("consts", [C, NJ + 2 * V], _F32, kind="ExternalInput").ap()
    # arow rows, packed: quad q -> partition (q%4)*32, cols (q//4)*512
    arowp_d = nc.dram_tensor("arowp", [4, (JQ // 4) * 512], _F32, kind="ExternalInput").ap()
    # output in bf16: tanh outputs are in [-1,1] so bf16 costs <=2e-3 abs
    # error, and it HALVES the store traffic (the kernel is store-bound).
    # The host upcasts to f32 after gather.
    out_d = nc.dram_tensor("out_shard", [S, NJ, V], _BF16, kind="ExternalOutput").ap()

    RW = (JQ // 4) * 512   # 8192 packed cols
    RCH = 2048             # f32r cast chunk (covers 2 jblocks)

    with tile.TileContext(nc) as tc, ExitStack() as ctx:
        singles = ctx.enter_context(tc.tile_pool(name="singles", bufs=1))
        rhs_pool = ctx.enter_context(tc.tile_pool(name="rhs", bufs=4))
        tmp_pool = ctx.enter_context(tc.tile_pool(name="tmp", bufs=2))
        psum_pool = ctx.enter_context(tc.tile_pool(name="psum", bufs=1, space="PSUM"))
        out_pool = ctx.enter_context(tc.tile_pool(name="outs", bufs=8))

        # ---- load constants, batched into 4 DMAs total: arowp chunk-0
        # rows (one 4-partition strided DMA) + ctxT on sync; consts pack +
        # arowp bulk on scalar ----
        arowp_sb = singles.tile([97, RW], _F32)
        consts_sb = singles.tile([C, NJ + 2 * V], _F32)
        nc.scalar.dma_start(out=consts_sb, in_=consts_d)
        for r in (0, 2):
            nc.sync.dma_start(
                out=arowp_sb[32 * r:32 * r + 1, 0:RCH], in_=arowp_d[r:r + 1, 0:RCH]
            )
        for r in (1, 3):
            nc.scalar.dma_start(
                out=arowp_sb[32 * r:32 * r + 1, 0:RCH], in_=arowp_d[r:r + 1, 0:RCH]
            )
        ctxjT_sb = consts_sb[:, 0:NJ]
        wmT_sb = consts_sb[:, NJ:NJ + V]
        w2dT_sb = consts_sb[:, NJ + V:NJ + 2 * V]
        ctxT_sb = singles.tile([C, S], _F32)
        nc.sync.dma_start(out=ctxT_sb, in_=ctxT_d)
        for r in range(4):
            eng = nc.sync if r % 2 == 0 else nc.scalar
            eng.dma_start(
                out=arowp_sb[32 * r:32 * r + 1, RCH:RW], in_=arowp_d[r:r + 1, RCH:RW]
            )

        # ---- fp32r rounding, ordered so the first bias/main mms unblock
        # earliest: ones -> arowp chunk 0 -> ctxT -> (preps) -> rest ----
        ones_f = singles.tile([97, 128], _F32)
        nc.vector.memset(ones_f, 1.0)
        ones_r = singles.tile([97, 128], _F32R)
        nc.vector.tensor_copy(ones_r, ones_f)
        arowp_r = singles.tile([97, RW], _F32R)
        nc.vector.tensor_copy(arowp_r[:, 0:RCH], arowp_sb[:, 0:RCH])
        # Main mms run in fp16: 16-bit moving operands stream 2 cols/cycle
        # (like bf16, ~2x fp32r) but with 10 mantissa bits the quantization
        # error is ~4x smaller than bf16 (total ~3-4e-3 vs 1.6e-2).
        # Bias mms stay f32r so arow/biases stay exact.
        ctxT_r = singles.tile([C, S], _FP16)
        nc.vector.tensor_copy(ctxT_r, ctxT_sb)

        # broadcast APs for half-jblock (16 j's) prep: wmT/w2dT repeat over
        # the j dim (step 0), ctxjT j scalars repeat over the v dim (step 0)
        wm_b16 = bass.AP(
            tensor=wmT_sb.tensor,
            offset=wmT_sb.offset,
            ap=[wmT_sb.ap[0], [0, 16], wmT_sb.ap[1]],
        )
        w2d_b16 = bass.AP(
            tensor=w2dT_sb.tensor,
            offset=w2dT_sb.offset,
            ap=[w2dT_sb.ap[0], [0, 16], w2dT_sb.ap[1]],
        )

        # one 8-bank psum megatile; bank b occupies [:, b*512:(b+1)*512]
        P = psum_pool.tile([128, 4096], _F32, name="mega")

        dma_engines = [nc.sync, nc.scalar]
        dma_i = 0

        def prep_half(jb, h):
            # rhs' for 16 j's (quads 8jb+4h .. +3): one mult + one add [C, 2048]
            j0 = 32 * jb + 16 * h
            tmp_p = tmp_pool.tile([C, 16 * V], _F32, name="tmp")
            ctxj_bc = bass.AP(
                tensor=ctxjT_sb.tensor,
                offset=ctxjT_sb.offset + j0,
                ap=[ctxjT_sb.ap[0], [1, 16], [0, V]],
            )
            nc.vector.tensor_tensor(
                out=tmp_p, in0=wm_b16, in1=ctxj_bc, op=mybir.AluOpType.mult
            )
            rhs_p = rhs_pool.tile([C, 16 * V], _FP16, name="rhs")
            nc.vector.tensor_tensor(
                out=rhs_p, in0=tmp_p, in1=w2d_b16, op=mybir.AluOpType.add
            )
            return rhs_p

        for jb in range(NJB):
            halves = [prep_half(jb, 0), prep_half(jb, 1)]
            if jb == 1:
                for cc in range(1, RW // RCH):
                    nc.vector.tensor_copy(
                        arowp_r[:, cc * RCH:(cc + 1) * RCH],
                        arowp_sb[:, cc * RCH:(cc + 1) * RCH],
                    )

            for ib in range(NIB):
                # ---- all 8 bias mms first (two strip-concurrent groups),
                # then all 8 main mms with a single ctxT LDW: long PE bursts
                # (fewer HAM re-throttles), 16x -> 5x fewer LDWEIGHTS ----
                for half in range(2):
                    for s in range(4):
                        strip = s * 32
                        col = (2 * jb + half) * 512
                        bank = 4 * half + s
                        nc.tensor.matmul(
                            P[:, bank * 512:(bank + 1) * 512],
                            lhsT=ones_r[strip:strip + 1, :],
                            rhs=arowp_r[strip:strip + 1, col:col + 512],
                            start=True,
                            stop=False,
                            tile_position=(strip, 0),
                        )
                for half in range(2):
                    for s in range(4):
                        bank = 4 * half + s
                        nc.tensor.matmul(
                            P[:, bank * 512:(bank + 1) * 512],
                            lhsT=ctxT_r[:, ib * 128:(ib + 1) * 128],
                            rhs=halves[half][:, s * 512:(s + 1) * 512],
                            start=False,
                            stop=True,
                        )
                    # ---- drain the half as soon as its 4 mains are done:
                    # tanh [128,2048] + ONE 1 MiB DMA, 8 KiB/partition
                    # contiguous ----
                    ot = out_pool.tile([128, 2048], _BF16, name="ot")
                    nc.scalar.activation(
                        ot, P[:, half * 2048:(half + 1) * 2048],
                        mybir.ActivationFunctionType.Tanh,
                    )
                    # ONE 512 KiB store per drained tile, alternating HWDGE
                    # rings so consecutive tiles overlap on the two rings
                    # (splitting a tile across both rings serializes tiles
                    # and measured ~30 us slower)
                    dst = bass.AP(
                        tensor=out_d.tensor,
                        offset=(ib * 128) * NJ * V + (32 * jb + 16 * half) * V,
                        ap=[[NJ * V, 128], [1, 16 * V]],
                    )
                    eng = dma_engines[dma_i % 2]
                    dma_i += 1
                    eng.dma_start(out=dst, in_=ot)

    nc.compile()
    return nc


_NC_CACHE = {}


def get_nc():
    if "nc" not in _NC_CACHE:
        _NC_CACHE["nc"] = build_nc()
    return _NC_CACHE["nc"]


def make_in_maps(ctx, W1, b1, W2, b2, Wm, bm, Wd, bd):
    ctx = np.asarray(ctx, np.float32)
    bias_all = (
        np.asarray(b1) + np.asarray(b2) + np.asarray(bm) + np.asarray(bd)
    ).astype(np.float32)
    wmT = np.ascontiguousarray(np.asarray(Wm, np.float32).T)                  # (C,V)
    w2dT = np.ascontiguousarray(
        (np.asarray(W2) - np.asarray(Wd)).T.astype(np.float32)
    )
    w1d = (np.asarray(W1) + np.asarray(Wd)).astype(np.float32)                # (V,C)

    in_maps = []
    for k in range(NCORES):
        b = k // 2
        jo = (k % 2) * NJ
        arow = (ctx[b, jo:jo + NJ] @ w1d.T + bias_all).astype(np.float32)     # (NJ,V)
        arowp = np.zeros((4, (JQ // 4) * 512), np.float32)
        arowq = arow.reshape(JQ, 512)                                          # quad rows
        for q in range(JQ):
            arowp[q % 4, (q // 4) * 512:(q // 4) * 512 + 512] = arowq[q]
        consts = np.concatenate(
            [np.ascontiguousarray(ctx[b, jo:jo + NJ].T), wmT, w2dT], axis=1
        ).astype(np.float32)
        in_maps.append({
            "ctxT": np.ascontiguousarray(ctx[b].T),
            "consts": np.ascontiguousarray(consts),
            "arowp": arowp,
        })
    return in_maps


def run(in_maps, **kw):
    return run_bass_kernel_spmd(get_nc(), in_maps, core_ids=list(range(NCORES)), **kw)


def assemble(results):
    out = np.empty((B, S, S, V), np.float32)
    for k in range(NCORES):
        b = k // 2
        jo = (k % 2) * NJ
        out[b, :, jo:jo + NJ] = np.asarray(results[k]["out_shard"], np.float32)
    return out


def kernel(ctx, W1, b1, W2, b2, Wm, bm, Wd, bd):
    install_ntff_shim()
    in_maps = make_in_maps(ctx, W1, b1, W2, b2, Wm, bm, Wd, bd)
    res = run(in_maps)
    return assemble(res.results)
